# revision 6
# baseline (speedup 1.0000x reference)
"""Luong local-p attention (scaled-dot, gaussian window) on 8 trn2 cores.

Strategy (data-parallel over batch, 2 examples/core):
  - Host: transpose source_hidden_states to [H, S] per example so the score
    matmul can contract over H on the TensorEngine with the target vector
    replicated as the stationary operand (scores come out replicated across
    all 128 partitions, which is exactly the layout the windowed context
    multiply needs).
  - Device per example:
      p = S*sigmoid(v_p . tanh(W_p^T t + b_p) + b_v)   (fp32 PE matmul + ACT)
      scores[s] = (src[s,:] . t) / sqrt(H)              (PE, psum-accumulated)
      softmax stats over full S (DVE max / ACT exp+accum / DVE sum)
      window [s0, s0+512), s0 = clamp(round(p)-256, 0, S-512) covers all
      positions where the gaussian factor is > ~1e-14; context is reduced
      over the window only (dynamic-offset DMA re-fetch of the window
      columns + fused multiply-reduce on DVE).
"""

import numpy as np

N_CORES = 8
B, S, H = 16, 4096, 1024
BEX = B // N_CORES  # examples per core
NH = H // 128  # h-chunks of 128 partitions
NSB = S // 512  # s-blocks of 512
WIN = 512
SCALE = 1.0 / 32.0  # 1/sqrt(H)
GEXP = -1.0 / 2048.0  # -1/(2*sigma^2), sigma = WINDOW/2 = 32
S0MAX = float(S - WIN)

_CACHE = {}


def _build():
    import concourse.bacc as bacc
    import concourse.bass as bass
    import concourse.mybir as mybir
    import concourse.tile as tile

    f32 = mybir.dt.float32
    i32 = mybir.dt.int32
    AF = mybir.ActivationFunctionType
    OP = mybir.AluOpType
    AX = mybir.AxisListType
    ds = bass.ds

    nc = bacc.Bacc("TRN2", target_bir_lowering=False, debug=False, num_devices=N_CORES)
    srcT = nc.dram_tensor("srcT", [BEX, H, S], f32, kind="ExternalInput").ap()
    tgt = nc.dram_tensor("tgt", [BEX, H], f32, kind="ExternalInput").ap()
    wp = nc.dram_tensor("wp", [H, H], f32, kind="ExternalInput").ap()
    vp = nc.dram_tensor("vp", [1, H], f32, kind="ExternalInput").ap()
    bp = nc.dram_tensor("bp", [1, H], f32, kind="ExternalInput").ap()
    bv = nc.dram_tensor("bv", [1, 1], f32, kind="ExternalInput").ap()
    out = nc.dram_tensor("out", [BEX, NH, 128], f32, kind="ExternalOutput").ap()
    scr_sp = nc.dram_tensor("scr_sp", [BEX, 1], f32).ap()

    with tile.TileContext(nc) as tc:
        with (
            tc.tile_pool(name="cpool", bufs=1) as cpool,
            tc.tile_pool(name="wpool", bufs=2) as wpool,
            tc.tile_pool(name="spool", bufs=4) as spool,
            tc.tile_pool(name="winpool", bufs=3) as winpool,
            tc.tile_pool(name="mpool", bufs=2) as mpool,
        ):
            # ---------------- phase 0: p = S*sigmoid(v . tanh(W^T t + b)) ----
            tT = []
            for c in range(NH):
                t_ = cpool.tile([128, BEX], f32, tag=f"tT{c}")
                nc.sync.dma_start(t_[:], tgt[0:BEX, c * 128 : (c + 1) * 128].transpose([1, 0]))
                tT.append(t_)

            bp_sb = cpool.tile([BEX, H], f32, tag="bp_sb")
            v_b = cpool.tile([BEX, H], f32, tag="v_b")
            bv_sb = cpool.tile([BEX, 1], f32, tag="bv_sb")
            for e in range(BEX):
                nc.sync.dma_start(bp_sb[e : e + 1, :], bp[0:1, :])
                nc.sync.dma_start(v_b[e : e + 1, :], vp[0:1, :])
                nc.sync.dma_start(bv_sb[e : e + 1, :], bv[0:1, :])

            with tc.tile_pool(name="psA", bufs=1, space="PSUM") as psA:
                ps_hp0 = psA.tile([BEX, 512], f32, tag="hp0")
                ps_hp1 = psA.tile([BEX, 512], f32, tag="hp1")
                for c in range(NH):
                    wt = wpool.tile([128, H], f32, tag="w")
                    nc.sync.dma_start(wt[:], wp[c * 128 : (c + 1) * 128, :])
                    nc.tensor.matmul(
                        ps_hp0[:], tT[c][:], wt[:, 0:512], start=(c == 0), stop=(c == NH - 1)
                    )
                    nc.tensor.matmul(
                        ps_hp1[:], tT[c][:], wt[:, 512:1024], start=(c == 0), stop=(c == NH - 1)
                    )

                hp_sb = cpool.tile([BEX, H], f32, tag="hp_sb")
                nc.vector.tensor_tensor(hp_sb[:, 0:512], ps_hp0[:], bp_sb[:, 0:512], OP.add)
                nc.vector.tensor_tensor(hp_sb[:, 512:1024], ps_hp1[:], bp_sb[:, 512:1024], OP.add)

            nc.scalar.activation(hp_sb[:], hp_sb[:], AF.Tanh)
            ttr_scr = cpool.tile([BEX, H], f32, tag="ttr_scr")
            pre = cpool.tile([BEX, 1], f32, tag="pre")
            nc.vector.tensor_tensor(ttr_scr[:], hp_sb[:], v_b[:], OP.mult)
            nc.vector.tensor_reduce(pre[:], ttr_scr[:], AX.X, OP.add)
            pv = cpool.tile([BEX, 1], f32, tag="pv")
            nc.scalar.activation(pv[:], pre[:], AF.Sigmoid, bias=bv_sb[:], scale=1.0)
            nc.vector.tensor_scalar(pv[:], pv[:], float(S), None, OP.mult)

            s0f = cpool.tile([BEX, 1], f32, tag="s0f")
            nc.vector.tensor_scalar(s0f[:], pv[:], 256.0, None, OP.subtract)
            nc.vector.tensor_scalar(s0f[:], s0f[:], 0.0, S0MAX, OP.max, OP.min)
            s0i = cpool.tile([BEX, 1], i32, tag="s0i")
            nc.vector.tensor_copy(s0i[:], s0f[:])
            s0ff = cpool.tile([BEX, 1], f32, tag="s0ff")
            nc.vector.tensor_copy(s0ff[:], s0i[:])

            spd = cpool.tile([BEX, 1], f32, tag="spd")
            nc.vector.tensor_tensor(spd[:], s0ff[:], pv[:], OP.subtract)
            nc.sync.dma_start(scr_sp[:], spd[:])

            s0_regs = []
            for e in range(BEX):
                s0_regs.append(
                    nc.values_load(
                        s0i[e : e + 1, 0:1],
                        min_val=0,
                        max_val=int(S0MAX),
                        skip_runtime_bounds_check=True,
                    )
                )

            # gaussian window factors per example: exp(-(s0 + f - p)^2 / (2 s^2))
            iota_i = cpool.tile([128, WIN], i32, tag="iota_i")
            nc.gpsimd.iota(iota_i[:], pattern=[[1, WIN]], base=0, channel_multiplier=0)
            iota_f = cpool.tile([128, WIN], f32, tag="iota_f")
            nc.vector.tensor_copy(iota_f[:], iota_i[:])

            gauss = []
            for e in range(BEX):
                sp_b = cpool.tile([128, 1], f32, tag=f"sp_b{e}")
                nc.sync.dma_start(sp_b[:], scr_sp[e : e + 1, 0:1].to_broadcast((128, 1)))
                d = mpool.tile([128, WIN], f32, tag="d")
                nc.vector.tensor_scalar(d[:], iota_f[:], sp_b[:], None, OP.add)
                d2 = mpool.tile([128, WIN], f32, tag="d2")
                nc.scalar.activation(d2[:], d[:], AF.Square)
                g = cpool.tile([128, WIN], f32, tag=f"gauss{e}")
                nc.scalar.activation(g[:], d2[:], AF.Exp, scale=GEXP)
                gauss.append(g)

            # replicated-target stationary operands: t_rep[e][c][k, m] = t[e, 128c + k]
            zeros = cpool.tile([128, 128], f32, tag="zeros")
            nc.vector.memset(zeros[:], 0.0)
            t_rep = []
            for e in range(BEX):
                reps = []
                for c in range(NH):
                    r = cpool.tile([128, 128], f32, tag=f"t_rep{e}_{c}")
                    nc.scalar.activation(
                        r[:], zeros[:], AF.Identity, bias=tT[c][:, e : e + 1], scale=1.0
                    )
                    reps.append(r)
                t_rep.append(reps)

            # ---------------- per-example: scores, softmax stats, context ----
            with tc.tile_pool(name="psB", bufs=1, space="PSUM") as psB:
                for e in range(BEX):
                    ps = [
                        psB.tile([128, 512], f32, tag=f"sc{k}", name=f"sc{k}_{e}")
                        for k in range(NSB)
                    ]
                    for c in range(NH):
                        big = spool.tile([128, S], f32, tag="stream")
                        nc.sync.dma_start(big[:], srcT[e, c * 128 : (c + 1) * 128, :])
                        for k in range(NSB):
                            nc.tensor.matmul(
                                ps[k][:],
                                t_rep[e][c][:],
                                big[:, k * 512 : (k + 1) * 512],
                                start=(c == 0),
                                stop=(c == NH - 1),
                            )

                    # softmax stats over full S (scores are replicated across rows)
                    mx8 = mpool.tile([128, NSB], f32, tag="mx8")
                    for k in range(NSB):
                        nc.vector.tensor_reduce(mx8[:, k : k + 1], ps[k][:], AX.X, OP.max)
                    m = mpool.tile([128, 1], f32, tag="m")
                    nc.vector.tensor_reduce(m[:], mx8[:], AX.X, OP.max)
                    bias_m = mpool.tile([128, 1], f32, tag="bias_m")
                    nc.vector.tensor_scalar(bias_m[:], m[:], -SCALE, None, OP.mult)

                    sums8 = mpool.tile([128, NSB], f32, tag="sums8")
                    expsc = mpool.tile([128, S], f32, tag=f"expsc{e}")
                    for k in range(NSB):
                        nc.scalar.activation(
                            expsc[:, k * 512 : (k + 1) * 512],
                            ps[k][:],
                            AF.Exp,
                            bias=bias_m[:],
                            scale=SCALE,
                            accum_out=sums8[:, k : k + 1],
                        )
                    z = mpool.tile([128, 1], f32, tag="z")
                    nc.vector.tensor_reduce(z[:], sums8[:], AX.X, OP.add)
                    rz = mpool.tile([128, 1], f32, tag="rz")
                    nc.vector.reciprocal(rz[:], z[:])

                    # windowed context
                    expw = mpool.tile([128, WIN], f32, tag="expw")
                    nc.vector.tensor_copy(expw[:], expsc[:, ds(s0_regs[e], WIN)])
                    attnw = mpool.tile([128, WIN], f32, tag="attnw")
                    nc.vector.tensor_tensor(attnw[:], expw[:], gauss[e][:], OP.mult)

                    ctx = mpool.tile([128, NH], f32, tag="ctx")
                    scr512 = mpool.tile([128, WIN], f32, tag="scr512")
                    for c in range(NH):
                        win = winpool.tile([128, WIN], f32, tag="win")
                        nc.sync.dma_start(
                            win[:], srcT[e, c * 128 : (c + 1) * 128, ds(s0_regs[e], WIN)]
                        )
                        nc.vector.tensor_tensor(scr512[:], win[:], attnw[:], OP.mult)
                        nc.vector.tensor_reduce(ctx[:, c : c + 1], scr512[:], AX.X, OP.add)
                    nc.vector.tensor_scalar(ctx[:], ctx[:], rz[:], None, OP.mult)
                    nc.sync.dma_start(out[e].transpose([1, 0]), ctx[:])

    nc.compile()
    return nc


def _get_nc():
    if "nc" not in _CACHE:
        _CACHE["nc"] = _build()
    return _CACHE["nc"]


def kernel(source_hidden_states, target_hidden_state, W_p, b_p, v_p, b_v):
    from concourse.bass_utils import run_bass_kernel_spmd

    src = np.asarray(source_hidden_states, dtype=np.float32)
    tgt = np.asarray(target_hidden_state, dtype=np.float32)
    wp = np.asarray(W_p, dtype=np.float32)
    bp = np.asarray(b_p, dtype=np.float32).reshape(1, H)
    vp = np.asarray(v_p, dtype=np.float32).reshape(1, H)
    bv = np.asarray(b_v, dtype=np.float32).reshape(1, 1)

    srcT = np.ascontiguousarray(src.transpose(0, 2, 1))  # [B, H, S]

    nc = _get_nc()
    in_maps = []
    for k in range(N_CORES):
        lo, hi = k * BEX, (k + 1) * BEX
        in_maps.append(
            {
                "srcT": srcT[lo:hi],
                "tgt": np.ascontiguousarray(tgt[lo:hi]),
                "wp": wp,
                "vp": vp,
                "bp": bp,
                "bv": bv,
            }
        )
    r = run_bass_kernel_spmd(nc, in_maps, list(range(N_CORES)))
    outs = [r.results[k]["out"].reshape(BEX, H) for k in range(N_CORES)]
    return np.concatenate(outs, axis=0)


# revision 9
# speedup vs baseline: 1.2222x; 1.2222x over previous
"""Luong local-p attention (scaled-dot, gaussian window) on 8 trn2 cores.

Strategy (data-parallel over batch, 2 examples/core):
  - Host: transpose source_hidden_states to [H, S] per example so the score
    matmul can contract over H on the TensorEngine with the target vector
    replicated as the stationary operand (scores come out replicated across
    all 128 partitions, which is exactly the layout the windowed context
    multiply needs). Ships a bf16 copy (streamed once for scores/softmax
    stats) and keeps the fp32 copy for the window re-read.
  - Device per example:
      p = S*sigmoid(v_p . tanh(W_p^T t + b_p) + b_v)   (fp32 PE matmul + ACT)
      scores[s] = (src[s,:] . t) / sqrt(H)              (bf16 PE, psum-acc)
      softmax max + denominator Z over full S from the bf16 scores (errors
      average out across ~10^3 effective terms)
      window [s0, s0+512), s0 = clamp(floor(p)-256, 0, S-512) covers every
      position whose gaussian factor is > ~1e-14; window scores are
      recomputed in fp32 from the re-fetched fp32 window columns, using the
      same max m as Z (m cancels in the softmax ratio), so the weights that
      matter are fp32-accurate. Context = windowed multiply-reduce on DVE.
"""

import numpy as np

N_CORES = 8
B, S, H = 16, 4096, 1024
BEX = B // N_CORES  # examples per core
NH = H // 128  # h-chunks of 128 partitions
NSB = S // 512  # s-blocks of 512
WIN = 512
SCALE = 1.0 / 32.0  # 1/sqrt(H)
GEXP = -1.0 / 2048.0  # -1/(2*sigma^2), sigma = WINDOW/2 = 32
S0MAX = float(S - WIN)

_CACHE = {}


def _build():
    import concourse.bacc as bacc
    import concourse.bass as bass
    import concourse.mybir as mybir
    import concourse.tile as tile

    f32 = mybir.dt.float32
    bf16 = mybir.dt.bfloat16
    i32 = mybir.dt.int32
    AF = mybir.ActivationFunctionType
    OP = mybir.AluOpType
    AX = mybir.AxisListType
    ds = bass.ds

    nc = bacc.Bacc("TRN2", target_bir_lowering=False, debug=False, num_devices=N_CORES)
    srcT = nc.dram_tensor("srcT", [BEX, H, S], f32, kind="ExternalInput").ap()
    srcTb = nc.dram_tensor("srcTb", [BEX, H, S], bf16, kind="ExternalInput").ap()
    tgt = nc.dram_tensor("tgt", [BEX, H], f32, kind="ExternalInput").ap()
    wp = nc.dram_tensor("wp", [H, H], f32, kind="ExternalInput").ap()
    vp = nc.dram_tensor("vp", [1, H], f32, kind="ExternalInput").ap()
    bp = nc.dram_tensor("bp", [1, H], f32, kind="ExternalInput").ap()
    bv = nc.dram_tensor("bv", [1, 1], f32, kind="ExternalInput").ap()
    out = nc.dram_tensor("out", [BEX, NH, 128], f32, kind="ExternalOutput").ap()
    scr_sp = nc.dram_tensor("scr_sp", [BEX, 1], f32).ap()

    with tile.TileContext(nc) as tc:
        with (
            tc.tile_pool(name="cpool", bufs=1) as cpool,
            tc.tile_pool(name="wpool", bufs=2) as wpool,
            tc.tile_pool(name="spool", bufs=4) as spool,
            tc.tile_pool(name="winpool", bufs=10) as winpool,
            tc.tile_pool(name="mpool", bufs=2) as mpool,
        ):
            # ---------------- phase 0: p = S*sigmoid(v . tanh(W^T t + b)) ----
            tT = []
            for c in range(NH):
                t_ = cpool.tile([128, BEX], f32, tag=f"tT{c}")
                nc.sync.dma_start(t_[:], tgt[0:BEX, c * 128 : (c + 1) * 128].transpose([1, 0]))
                tT.append(t_)

            bp_sb = cpool.tile([BEX, H], f32, tag="bp_sb")
            v_b = cpool.tile([BEX, H], f32, tag="v_b")
            bv_sb = cpool.tile([BEX, 1], f32, tag="bv_sb")
            for e in range(BEX):
                nc.sync.dma_start(bp_sb[e : e + 1, :], bp[0:1, :])
                nc.sync.dma_start(v_b[e : e + 1, :], vp[0:1, :])
                nc.sync.dma_start(bv_sb[e : e + 1, :], bv[0:1, :])

            with tc.tile_pool(name="psA", bufs=1, space="PSUM") as psA:
                ps_hp0 = psA.tile([BEX, 512], f32, tag="hp0")
                ps_hp1 = psA.tile([BEX, 512], f32, tag="hp1")
                for c in range(NH):
                    wt = wpool.tile([128, H], f32, tag="w")
                    nc.sync.dma_start(wt[:], wp[c * 128 : (c + 1) * 128, :])
                    nc.tensor.matmul(
                        ps_hp0[:], tT[c][:], wt[:, 0:512], start=(c == 0), stop=(c == NH - 1)
                    )
                    nc.tensor.matmul(
                        ps_hp1[:], tT[c][:], wt[:, 512:1024], start=(c == 0), stop=(c == NH - 1)
                    )

                hp_sb = cpool.tile([BEX, H], f32, tag="hp_sb")
                nc.vector.tensor_tensor(hp_sb[:, 0:512], ps_hp0[:], bp_sb[:, 0:512], OP.add)
                nc.vector.tensor_tensor(hp_sb[:, 512:1024], ps_hp1[:], bp_sb[:, 512:1024], OP.add)

            nc.scalar.activation(hp_sb[:], hp_sb[:], AF.Tanh)
            ttr_scr = cpool.tile([BEX, H], f32, tag="ttr_scr")
            pre = cpool.tile([BEX, 1], f32, tag="pre")
            nc.vector.tensor_tensor(ttr_scr[:], hp_sb[:], v_b[:], OP.mult)
            nc.vector.tensor_reduce(pre[:], ttr_scr[:], AX.X, OP.add)
            pv = cpool.tile([BEX, 1], f32, tag="pv")
            nc.scalar.activation(pv[:], pre[:], AF.Sigmoid, bias=bv_sb[:], scale=1.0)
            nc.vector.tensor_scalar(pv[:], pv[:], float(S), None, OP.mult)

            s0f = cpool.tile([BEX, 1], f32, tag="s0f")
            nc.vector.tensor_scalar(s0f[:], pv[:], 256.0, None, OP.subtract)
            nc.vector.tensor_scalar(s0f[:], s0f[:], 0.0, S0MAX, OP.max, OP.min)
            s0i = cpool.tile([BEX, 1], i32, tag="s0i")
            nc.vector.tensor_copy(s0i[:], s0f[:])
            s0ff = cpool.tile([BEX, 1], f32, tag="s0ff")
            nc.vector.tensor_copy(s0ff[:], s0i[:])

            spd = cpool.tile([BEX, 1], f32, tag="spd")
            nc.vector.tensor_tensor(spd[:], s0ff[:], pv[:], OP.subtract)
            nc.sync.dma_start(scr_sp[:], spd[:])

            s0_regs = []
            for e in range(BEX):
                s0_regs.append(
                    nc.values_load(
                        s0i[e : e + 1, 0:1],
                        min_val=0,
                        max_val=int(S0MAX),
                        skip_runtime_bounds_check=True,
                    )
                )

            # gaussian window factors per example: exp(-(s0 + f - p)^2 / (2 s^2))
            iota_i = cpool.tile([128, WIN], i32, tag="iota_i")
            nc.gpsimd.iota(iota_i[:], pattern=[[1, WIN]], base=0, channel_multiplier=0)
            iota_f = cpool.tile([128, WIN], f32, tag="iota_f")
            nc.vector.tensor_copy(iota_f[:], iota_i[:])

            gauss = []
            for e in range(BEX):
                sp_b = cpool.tile([128, 1], f32, tag=f"sp_b{e}")
                nc.sync.dma_start(sp_b[:], scr_sp[e : e + 1, 0:1].to_broadcast((128, 1)))
                d = mpool.tile([128, WIN], f32, tag="d")
                nc.vector.tensor_scalar(d[:], iota_f[:], sp_b[:], None, OP.add)
                d2 = mpool.tile([128, WIN], f32, tag="d2")
                nc.scalar.activation(d2[:], d[:], AF.Square)
                g = cpool.tile([128, WIN], f32, tag=f"gauss{e}")
                nc.scalar.activation(g[:], d2[:], AF.Exp, scale=GEXP)
                gauss.append(g)

            # replicated-target stationary operands: t_rep[e][c][k, m] = t[e, 128c + k]
            zeros = cpool.tile([128, 128], f32, tag="zeros")
            nc.vector.memset(zeros[:], 0.0)
            t_rep32 = []
            t_rep16 = []
            for e in range(BEX):
                r32s, r16s = [], []
                for c in range(NH):
                    r32 = cpool.tile([128, 128], f32, tag=f"t_rep32_{e}_{c}")
                    nc.scalar.activation(
                        r32[:], zeros[:], AF.Identity, bias=tT[c][:, e : e + 1], scale=1.0
                    )
                    r16 = cpool.tile([128, 128], bf16, tag=f"t_rep16_{e}_{c}")
                    nc.vector.tensor_copy(r16[:], r32[:])
                    r32s.append(r32)
                    r16s.append(r16)
                t_rep32.append(r32s)
                t_rep16.append(r16s)

            # ---------------- per-example: scores, softmax stats, context ----
            with tc.tile_pool(name="psB", bufs=1, space="PSUM") as psB:
                for e in range(BEX):
                    ps = [
                        psB.tile([128, 512], f32, tag=f"sc{k}", name=f"sc{k}_{e}")
                        for k in range(NSB)
                    ]
                    for c in range(NH):
                        big = spool.tile([128, S], bf16, tag="stream")
                        nc.sync.dma_start(big[:], srcTb[e, c * 128 : (c + 1) * 128, :])
                        for k in range(NSB):
                            nc.tensor.matmul(
                                ps[k][:],
                                t_rep16[e][c][:],
                                big[:, k * 512 : (k + 1) * 512],
                                start=(c == 0),
                                stop=(c == NH - 1),
                            )

                    # softmax stats over full S (scores are replicated across rows)
                    mx8 = mpool.tile([128, NSB], f32, tag="mx8")
                    for k in range(NSB):
                        nc.vector.tensor_reduce(mx8[:, k : k + 1], ps[k][:], AX.X, OP.max)
                    m = mpool.tile([128, 1], f32, tag="m")
                    nc.vector.tensor_reduce(m[:], mx8[:], AX.X, OP.max)
                    bias_m = mpool.tile([128, 1], f32, tag="bias_m")
                    nc.vector.tensor_scalar(bias_m[:], m[:], -SCALE, None, OP.mult)

                    sums8 = mpool.tile([128, NSB], f32, tag="sums8")
                    for k in range(NSB):
                        ej = mpool.tile([128, 512], f32, tag="expjunk")
                        nc.scalar.activation(
                            ej[:],
                            ps[k][:],
                            AF.Exp,
                            bias=bias_m[:],
                            scale=SCALE,
                            accum_out=sums8[:, k : k + 1],
                        )
                    z = mpool.tile([128, 1], f32, tag="z")
                    nc.vector.tensor_reduce(z[:], sums8[:], AX.X, OP.add)
                    rz = mpool.tile([128, 1], f32, tag="rz")
                    nc.vector.reciprocal(rz[:], z[:])

                    # fp32 window: re-fetch window columns, recompute scores in
                    # fp32 (same max m cancels against Z), then context.
                    wins = []
                    psw = psB.tile([128, 512], f32, tag="sc7", name=f"win_ps_{e}")
                    for c in range(NH):
                        win = winpool.tile([128, WIN], f32, tag="win", name=f"win_{e}_{c}")
                        nc.sync.dma_start(
                            win[:], srcT[e, c * 128 : (c + 1) * 128, ds(s0_regs[e], WIN)]
                        )
                        wins.append(win)
                        nc.tensor.matmul(
                            psw[:],
                            t_rep32[e][c][:],
                            win[:],
                            start=(c == 0),
                            stop=(c == NH - 1),
                        )

                    expw = mpool.tile([128, WIN], f32, tag="expw")
                    nc.scalar.activation(expw[:], psw[:], AF.Exp, bias=bias_m[:], scale=SCALE)
                    attnw = mpool.tile([128, WIN], f32, tag="attnw")
                    nc.vector.tensor_tensor(attnw[:], expw[:], gauss[e][:], OP.mult)

                    ctx = mpool.tile([128, NH], f32, tag="ctx")
                    scr512 = mpool.tile([128, WIN], f32, tag="scr512")
                    for c in range(NH):
                        nc.vector.tensor_tensor(scr512[:], wins[c][:], attnw[:], OP.mult)
                        nc.vector.tensor_reduce(ctx[:, c : c + 1], scr512[:], AX.X, OP.add)
                    nc.vector.tensor_scalar(ctx[:], ctx[:], rz[:], None, OP.mult)
                    nc.sync.dma_start(out[e].transpose([1, 0]), ctx[:])

    nc.compile()
    return nc


def _get_nc():
    if "nc" not in _CACHE:
        _CACHE["nc"] = _build()
    return _CACHE["nc"]


def _make_in_maps(src, tgt, wp, bp, vp, bv):
    import ml_dtypes

    srcT = np.ascontiguousarray(src.transpose(0, 2, 1))  # [B, H, S]
    srcTb = srcT.astype(ml_dtypes.bfloat16)
    in_maps = []
    for k in range(N_CORES):
        lo, hi = k * BEX, (k + 1) * BEX
        in_maps.append(
            {
                "srcT": srcT[lo:hi],
                "srcTb": srcTb[lo:hi],
                "tgt": np.ascontiguousarray(tgt[lo:hi]),
                "wp": wp,
                "vp": vp,
                "bp": bp,
                "bv": bv,
            }
        )
    return in_maps


def kernel(source_hidden_states, target_hidden_state, W_p, b_p, v_p, b_v):
    from concourse.bass_utils import run_bass_kernel_spmd

    src = np.asarray(source_hidden_states, dtype=np.float32)
    tgt = np.asarray(target_hidden_state, dtype=np.float32)
    wp = np.asarray(W_p, dtype=np.float32)
    bp = np.asarray(b_p, dtype=np.float32).reshape(1, H)
    vp = np.asarray(v_p, dtype=np.float32).reshape(1, H)
    bv = np.asarray(b_v, dtype=np.float32).reshape(1, 1)

    nc = _get_nc()
    in_maps = _make_in_maps(src, tgt, wp, bp, vp, bv)
    r = run_bass_kernel_spmd(nc, in_maps, list(range(N_CORES)))
    outs = [r.results[k]["out"].reshape(BEX, H) for k in range(N_CORES)]
    return np.concatenate(outs, axis=0)


# revision 10
# speedup vs baseline: 1.3713x; 1.1220x over previous
"""Luong local-p attention (scaled-dot, gaussian window) on 8 trn2 cores.

Strategy (data-parallel over batch, 2 examples/core):
  - Host: transpose source_hidden_states to [H, S] per example so the score
    matmul can contract over H on the TensorEngine with the target vector
    replicated as the stationary operand (scores come out replicated across
    all 128 partitions, which is exactly the layout the windowed context
    multiply needs). Ships a bf16 copy (streamed once for scores/softmax
    stats) and keeps the fp32 copy for the window re-read.
  - Device per example:
      p = S*sigmoid(v_p . tanh(W_p^T t + b_p) + b_v)   (fp32 PE matmul + ACT)
      scores[s] = (src[s,:] . t) / sqrt(H)              (bf16 PE, psum-acc)
      softmax max + denominator Z over full S from the bf16 scores (errors
      average out across ~10^3 effective terms)
      window [s0, s0+512), s0 = clamp(floor(p)-256, 0, S-512) covers every
      position whose gaussian factor is > ~1e-14; window scores are
      recomputed in fp32 from the re-fetched fp32 window columns, using the
      same max m as Z (m cancels in the softmax ratio), so the weights that
      matter are fp32-accurate. Context = windowed multiply-reduce spread
      across GPSIMD/DVE (multiplies) and ACT/DVE (reductions).
  - Scheduling: example 0's scores stream first; the p-computation runs
    after them (reusing two freed psum banks) while example 1 streams; the
    dynamic-window register loads only on the SP engine so no compute
    engine stalls on them.
"""

import numpy as np

N_CORES = 8
B, S, H = 16, 4096, 1024
BEX = B // N_CORES  # examples per core
NH = H // 128  # h-chunks of 128 partitions
NSB = S // 512  # s-blocks of 512
WIN = 512
SCALE = 1.0 / 32.0  # 1/sqrt(H)
GEXP = -1.0 / 2048.0  # -1/(2*sigma^2), sigma = WINDOW/2 = 32
S0MAX = float(S - WIN)

_CACHE = {}


def _build():
    import concourse.bacc as bacc
    import concourse.bass as bass
    import concourse.mybir as mybir
    import concourse.tile as tile

    f32 = mybir.dt.float32
    bf16 = mybir.dt.bfloat16
    i32 = mybir.dt.int32
    AF = mybir.ActivationFunctionType
    OP = mybir.AluOpType
    AX = mybir.AxisListType
    ET = mybir.EngineType
    ds = bass.ds

    nc = bacc.Bacc("TRN2", target_bir_lowering=False, debug=False, num_devices=N_CORES)
    srcT = nc.dram_tensor("srcT", [BEX, H, S], f32, kind="ExternalInput").ap()
    srcTb = nc.dram_tensor("srcTb", [BEX, H, S], bf16, kind="ExternalInput").ap()
    tgt = nc.dram_tensor("tgt", [BEX, H], f32, kind="ExternalInput").ap()
    wp = nc.dram_tensor("wp", [H, H], f32, kind="ExternalInput").ap()
    vp = nc.dram_tensor("vp", [1, H], f32, kind="ExternalInput").ap()
    bp = nc.dram_tensor("bp", [1, H], f32, kind="ExternalInput").ap()
    bv = nc.dram_tensor("bv", [1, 1], f32, kind="ExternalInput").ap()
    out = nc.dram_tensor("out", [BEX, NH, 128], f32, kind="ExternalOutput").ap()
    scr_sp = nc.dram_tensor("scr_sp", [BEX, 1], f32).ap()

    with tile.TileContext(nc) as tc:
        with (
            tc.tile_pool(name="cpool", bufs=1) as cpool,
            tc.tile_pool(name="wpool", bufs=2) as wpool,
            tc.tile_pool(name="spool", bufs=6) as spool,
            tc.tile_pool(name="winpool", bufs=10) as winpool,
            tc.tile_pool(name="mpool", bufs=2) as mpool,
            tc.tile_pool(name="psB", bufs=1, space="PSUM") as psB,
        ):
            # ---------------- setup: tiny DMAs + stationary operands ---------
            tT = []
            for c in range(NH):
                t_ = cpool.tile([128, BEX], f32, tag=f"tT{c}")
                nc.sync.dma_start(t_[:], tgt[0:BEX, c * 128 : (c + 1) * 128].transpose([1, 0]))
                tT.append(t_)

            bp_sb = cpool.tile([BEX, H], f32, tag="bp_sb")
            v_b = cpool.tile([BEX, H], f32, tag="v_b")
            bv_sb = cpool.tile([BEX, 1], f32, tag="bv_sb")
            for e in range(BEX):
                nc.sync.dma_start(bp_sb[e : e + 1, :], bp[0:1, :])
                nc.sync.dma_start(v_b[e : e + 1, :], vp[0:1, :])
                nc.sync.dma_start(bv_sb[e : e + 1, :], bv[0:1, :])

            zeros = cpool.tile([128, 128], f32, tag="zeros")
            nc.vector.memset(zeros[:], 0.0)
            t_rep32 = []
            t_rep16 = []
            for e in range(BEX):
                r32s, r16s = [], []
                for c in range(NH):
                    r32 = cpool.tile([128, 128], f32, tag=f"t_rep32_{e}_{c}")
                    nc.scalar.activation(
                        r32[:], zeros[:], AF.Identity, bias=tT[c][:, e : e + 1], scale=1.0
                    )
                    r16 = cpool.tile([128, 128], bf16, tag=f"t_rep16_{e}_{c}")
                    nc.vector.tensor_copy(r16[:], r32[:])
                    r32s.append(r32)
                    r16s.append(r16)
                t_rep32.append(r32s)
                t_rep16.append(r16s)

            iota_i = cpool.tile([128, WIN], i32, tag="iota_i")
            nc.gpsimd.iota(iota_i[:], pattern=[[1, WIN]], base=0, channel_multiplier=0)
            iota_f = cpool.tile([128, WIN], f32, tag="iota_f")
            nc.vector.tensor_copy(iota_f[:], iota_i[:])

            # W_p chunk DMAs (issued early; consumed by phase-0 matmuls later)
            wts = []
            for c in range(NH):
                wt = wpool.tile([128, H], f32, tag="w", bufs=8, name=f"wt{c}")
                nc.sync.dma_start(wt[:], wp[c * 128 : (c + 1) * 128, :])
                wts.append(wt)

            def scores_phase(e):
                ps = [
                    psB.tile([128, 512], f32, tag=f"sc{k}", name=f"sc{k}_{e}")
                    for k in range(NSB)
                ]
                for c in range(NH):
                    big = spool.tile([128, S], bf16, tag="stream", name=f"big_{e}_{c}")
                    nc.sync.dma_start(big[:], srcTb[e, c * 128 : (c + 1) * 128, :])
                    for k in range(NSB):
                        nc.tensor.matmul(
                            ps[k][:],
                            t_rep16[e][c][:],
                            big[:, k * 512 : (k + 1) * 512],
                            start=(c == 0),
                            stop=(c == NH - 1),
                        )
                return ps

            def stats_phase(e, ps):
                # softmax stats over full S (scores are replicated across rows)
                mx8 = mpool.tile([128, NSB], f32, tag="mx8", name=f"mx8_{e}")
                for k in range(NSB):
                    nc.vector.tensor_reduce(mx8[:, k : k + 1], ps[k][:], AX.X, OP.max)
                m = mpool.tile([128, 1], f32, tag="m", name=f"m_{e}")
                nc.vector.tensor_reduce(m[:], mx8[:], AX.X, OP.max)
                bias_m = mpool.tile([128, 1], f32, tag="bias_m", name=f"bias_m_{e}", bufs=2)
                nc.vector.tensor_scalar(bias_m[:], m[:], -SCALE, None, OP.mult)

                sums8 = mpool.tile([128, NSB], f32, tag="sums8", name=f"sums8_{e}")
                for k in range(NSB):
                    ej = mpool.tile([128, 512], f32, tag="expjunk", name=f"ej_{e}_{k}")
                    nc.scalar.activation(
                        ej[:],
                        ps[k][:],
                        AF.Exp,
                        bias=bias_m[:],
                        scale=SCALE,
                        accum_out=sums8[:, k : k + 1],
                    )
                z = mpool.tile([128, 1], f32, tag="z", name=f"z_{e}")
                nc.vector.tensor_reduce(z[:], sums8[:], AX.X, OP.add)
                rz = mpool.tile([128, 1], f32, tag="rz", name=f"rz_{e}", bufs=2)
                nc.vector.reciprocal(rz[:], z[:])
                return bias_m, rz

            def window_phase(e, s0_reg, gauss_e, bias_m, rz):
                # fp32 window: re-fetch window columns, recompute scores in
                # fp32 (same max m cancels against Z), then context.
                wins = []
                psw = psB.tile([128, 512], f32, tag="sc7", name=f"win_ps_{e}")
                for c in range(NH):
                    win = winpool.tile([128, WIN], f32, tag="win", name=f"win_{e}_{c}")
                    nc.sync.dma_start(
                        win[:], srcT[e, c * 128 : (c + 1) * 128, ds(s0_reg, WIN)]
                    )
                    wins.append(win)
                    nc.tensor.matmul(
                        psw[:], t_rep32[e][c][:], win[:], start=(c == 0), stop=(c == NH - 1)
                    )

                expw = mpool.tile([128, WIN], f32, tag="expw", name=f"expw_{e}")
                nc.scalar.activation(expw[:], psw[:], AF.Exp, bias=bias_m[:], scale=SCALE)
                attnw = mpool.tile([128, WIN], f32, tag="attnw", name=f"attnw_{e}")
                nc.vector.tensor_tensor(attnw[:], expw[:], gauss_e[:], OP.mult)

                ctx = mpool.tile([128, NH], f32, tag="ctx", name=f"ctx_{e}")
                for c in range(NH):
                    scr = mpool.tile(
                        [128, WIN], f32, tag="scr512", name=f"scr_{e}_{c}", bufs=4
                    )
                    if c % 2 == 0:
                        nc.vector.tensor_tensor(scr[:], wins[c][:], attnw[:], OP.mult)
                    else:
                        nc.gpsimd.tensor_tensor(scr[:], wins[c][:], attnw[:], OP.mult)
                    if c % 2 == 0:
                        ejc = mpool.tile(
                            [128, WIN], f32, tag="ctxjunk", name=f"cj_{e}_{c}", bufs=2
                        )
                        nc.scalar.activation(
                            ejc[:], scr[:], AF.Identity, accum_out=ctx[:, c : c + 1]
                        )
                    else:
                        nc.vector.tensor_reduce(ctx[:, c : c + 1], scr[:], AX.X, OP.add)
                nc.vector.tensor_scalar(ctx[:], ctx[:], rz[:], None, OP.mult)
                nc.sync.dma_start(out[e].transpose([1, 0]), ctx[:])

            # ---------------- example 0 scores (streams immediately) ---------
            ps0 = scores_phase(0)
            st0 = stats_phase(0, ps0)

            # ---------------- phase 0: p = S*sigmoid(v . tanh(W^T t + b)) ----
            ps_hp0 = psB.tile([BEX, 512], f32, tag="sc0", name="hp0")
            ps_hp1 = psB.tile([BEX, 512], f32, tag="sc1", name="hp1")
            for c in range(NH):
                nc.tensor.matmul(
                    ps_hp0[:], tT[c][:], wts[c][:, 0:512], start=(c == 0), stop=(c == NH - 1)
                )
                nc.tensor.matmul(
                    ps_hp1[:], tT[c][:], wts[c][:, 512:1024], start=(c == 0), stop=(c == NH - 1)
                )

            hp_sb = cpool.tile([BEX, H], f32, tag="hp_sb")
            nc.vector.tensor_tensor(hp_sb[:, 0:512], ps_hp0[:], bp_sb[:, 0:512], OP.add)
            nc.vector.tensor_tensor(hp_sb[:, 512:1024], ps_hp1[:], bp_sb[:, 512:1024], OP.add)

            nc.scalar.activation(hp_sb[:], hp_sb[:], AF.Tanh)
            ttr_scr = cpool.tile([BEX, H], f32, tag="ttr_scr")
            pre = cpool.tile([BEX, 1], f32, tag="pre")
            nc.vector.tensor_tensor(ttr_scr[:], hp_sb[:], v_b[:], OP.mult)
            nc.vector.tensor_reduce(pre[:], ttr_scr[:], AX.X, OP.add)
            pv = cpool.tile([BEX, 1], f32, tag="pv")
            nc.scalar.activation(pv[:], pre[:], AF.Sigmoid, bias=bv_sb[:], scale=1.0)
            nc.vector.tensor_scalar(pv[:], pv[:], float(S), None, OP.mult)

            s0f = cpool.tile([BEX, 1], f32, tag="s0f")
            nc.vector.tensor_scalar(s0f[:], pv[:], 256.0, None, OP.subtract)
            nc.vector.tensor_scalar(s0f[:], s0f[:], 0.0, S0MAX, OP.max, OP.min)
            s0i = cpool.tile([BEX, 1], i32, tag="s0i")
            nc.vector.tensor_copy(s0i[:], s0f[:])
            s0ff = cpool.tile([BEX, 1], f32, tag="s0ff")
            nc.vector.tensor_copy(s0ff[:], s0i[:])

            spd = cpool.tile([BEX, 1], f32, tag="spd")
            nc.vector.tensor_tensor(spd[:], s0ff[:], pv[:], OP.subtract)
            nc.sync.dma_start(scr_sp[:], spd[:])

            s0_regs = []
            for e in range(BEX):
                s0_regs.append(
                    nc.values_load(
                        s0i[e : e + 1, 0:1],
                        engines=[ET.SP],
                        min_val=0,
                        max_val=int(S0MAX),
                        skip_runtime_bounds_check=True,
                    )
                )

            # gaussian window factors per example: exp(-(s0 + f - p)^2 / (2 s^2))
            gauss = []
            for e in range(BEX):
                sp_b = cpool.tile([128, 1], f32, tag=f"sp_b{e}")
                nc.sync.dma_start(sp_b[:], scr_sp[e : e + 1, 0:1].to_broadcast((128, 1)))
                d = mpool.tile([128, WIN], f32, tag="d", name=f"d_{e}")
                nc.vector.tensor_scalar(d[:], iota_f[:], sp_b[:], None, OP.add)
                d2 = mpool.tile([128, WIN], f32, tag="d2", name=f"d2_{e}")
                nc.scalar.activation(d2[:], d[:], AF.Square)
                g = cpool.tile([128, WIN], f32, tag=f"gauss{e}")
                nc.scalar.activation(g[:], d2[:], AF.Exp, scale=GEXP)
                gauss.append(g)

            # ---------------- windows + second example ----------------------
            window_phase(0, s0_regs[0], gauss[0], *st0)
            ps1 = scores_phase(1)
            st1 = stats_phase(1, ps1)
            window_phase(1, s0_regs[1], gauss[1], *st1)

    nc.compile()
    return nc


def _get_nc():
    if "nc" not in _CACHE:
        _CACHE["nc"] = _build()
    return _CACHE["nc"]


def _make_in_maps(src, tgt, wp, bp, vp, bv):
    import ml_dtypes

    srcT = np.ascontiguousarray(src.transpose(0, 2, 1))  # [B, H, S]
    srcTb = srcT.astype(ml_dtypes.bfloat16)
    in_maps = []
    for k in range(N_CORES):
        lo, hi = k * BEX, (k + 1) * BEX
        in_maps.append(
            {
                "srcT": srcT[lo:hi],
                "srcTb": srcTb[lo:hi],
                "tgt": np.ascontiguousarray(tgt[lo:hi]),
                "wp": wp,
                "vp": vp,
                "bp": bp,
                "bv": bv,
            }
        )
    return in_maps


def kernel(source_hidden_states, target_hidden_state, W_p, b_p, v_p, b_v):
    from concourse.bass_utils import run_bass_kernel_spmd

    src = np.asarray(source_hidden_states, dtype=np.float32)
    tgt = np.asarray(target_hidden_state, dtype=np.float32)
    wp = np.asarray(W_p, dtype=np.float32)
    bp = np.asarray(b_p, dtype=np.float32).reshape(1, H)
    vp = np.asarray(v_p, dtype=np.float32).reshape(1, H)
    bv = np.asarray(b_v, dtype=np.float32).reshape(1, 1)

    nc = _get_nc()
    in_maps = _make_in_maps(src, tgt, wp, bp, vp, bv)
    r = run_bass_kernel_spmd(nc, in_maps, list(range(N_CORES)))
    outs = [r.results[k]["out"].reshape(BEX, H) for k in range(N_CORES)]
    return np.concatenate(outs, axis=0)
